# revision 2
# baseline (speedup 1.0000x reference)
"""AttentionWeightedAverage distributed Trainium2 kernel.

Reference computation (all f32):
    s     = wv @ v + wg @ h          # (512, 384) + (512, 1) broadcast
    t     = tanh(s)                  # (512, 384)
    z     = wh @ t                   # (384, 384)
    alpha = softmax(z, axis=-1)      # (384, 384)
    out[i, j, l] = v[j, l] * alpha[i, j]   # (384, 384, 384)

The output (226 MB f32) dwarfs the inputs (~2.5 MB), so the kernel is
bound by per-core HBM write bandwidth. Sharding: every core gets the
full (small) weights and computes s/t redundantly; core m owns rows
i in [m*48, (m+1)*48) of z/alpha and writes that contiguous slice of
the output. No collectives.

Measured structure of a run (NTFF profile, exec_time = end of NEFF
postamble minus first useful instruction):
- ~1 us framework preamble inside the window (const memsets + engine
  barrier) - fixed.
- input loads at ~318 GB/s combined across the two HWDGE queues.
- compute chain wv@v -> tanh -> z -> softmax -> alphaT.
- ~38.6 us bf16 store stream at ~385 GB/s (the roofline).
- ~7.7 us fixed postamble: the NEFF epilogue zeroes all 253
  semaphores individually across the 5 engines - compiler-generated,
  not controllable from kernel code.

Design (v2; v1 was ~63.5 us, see git history of this docstring):
- Output stored as bf16, upcast to f32 on host (gate is 2e-2
  scale-relative; measured total err 5.7e-3). 14.2 MB/core.
- v3 broadcast source uses layout B (v3[p, c*384+l] = v[3p+c, l]) so
  each partition's 3 output rows are consecutive -> 2304 B contiguous
  HBM runs per store descriptor row.
- All weights bf16 (fp8 on anything feeding z fails the gate: sharp
  softmax amplifies logit noise - measured 6e-2).
- Load order: wvb chunks (the s critical path) go on the SCALAR ring,
  which the SDMA arbitration served first in every baseline trace;
  hwg/whT/v3 go on sync. In the baseline (wvb on sync) wvb's first
  byte trailed the scalar queue by ~2 us, putting wvb k2 at 13.6 us;
  swapped, k2 lands ~11 us.
- PE warmups: one 128x128 memset then NWARM 128-wide throwaway
  matmuls issued immediately at body start. The HAM clock gate needs
  ~3.4 us of sustained PE activity to double the PE clock and
  re-throttles after a ~2.3 us gap; the baseline's 7 warmups ended
  1.2 us before wvb k0 landed and the whole s chain ran at 1.2 GHz
  (throttle_active 46 us). Warmups sized to bridge until k0 lands.
- z/softmax in ONE 48-row pass (ZSPLIT=1): closes only ~0.2 us later
  than a 24-row half (matmul cost is rhs-width-bound) and removes the
  mid-stream half-1 softmax interruption entirely (the baseline spent
  real tuning effort placing it; not needed at all).
- softmax skips the max-subtraction: |z| stays far from f32 exp
  overflow and softmax is shift-invariant. The exp's accum_out gives
  the row sums for free.
- ALL store dma_starts on the sync ring as uniform 2-row blocks
  (block 0 is 1 row so its descgen is ~0.7 us). ACT runs c1 of rows
  0..ACT_C1_ROWS plus every 4th middle row; everything else on DVE
  (~229 ns per 128x384 bf16 tensor_scalar at 2x mode - the [P,1] f32
  scalar blocks 4x).
- Run-to-run NEFF time varies with HBM contention from the neighbor
  core (per-engine SDMA queue backlog can extend the tail; not
  fixable from the kernel).

Per-core SBUF layouts (P = 128 partitions):
    wvb  (128, 3*896) bf16: per k: [wvT_k | vb_k];
         wvT_k[p, e] = wv[e, k*128+p], vb_k[p, l] = v[k*128+p, l]
    hwg  (128, 4+2048) bf16: [h3 | wgT3]; h3[p,k] = h[k*128+p],
         wgT3[p, k*512+e] = wg[e, k*128+p]
    whT3 (128, 192) bf16: whT3[p, k*48+i] = wh[m*48+i, k*128+p]
    v3   (128, 1152) bf16: v3[p, c*384+l] = v[3p+c, l]
"""

import numpy as np

import concourse.bacc as bacc
import concourse.mybir as mybir
from concourse import masks
from concourse.bass_utils import run_bass_kernel_spmd
from concourse.tile import TileContext

F32 = mybir.dt.float32
BF16 = mybir.dt.bfloat16
AF = mybir.ActivationFunctionType

NCORES = 8
L = 384          # vfeat_len == vfeat_dim
E = 512          # embed dim
IPC = L // NCORES  # 48 output rows per core
P = 128
CJ = L // P      # 3 chunks over the j axis
KV = L // P      # 3 contraction chunks for wv@v
KE = E // P      # 4 contraction chunks over embed dim
WVB = E + L      # fused [wvT_k | vb_k] chunk width
IPB = 2          # output rows batched per store DMA
OUT_BUFS = 8     # in-flight output tiles
HZ = IPC // 2    # z/softmax half size (ZSPLIT=2 path)
NWARM = 13       # 128-wide PE warmups (bridge body start -> wvb k0 lands)
ZSPLIT = 1       # 1: one 48-row z/softmax pass; 2: legacy halves
ZH1_AT = 13      # (ZSPLIT=2 only) emit z half 1 before this block row
ACT_C1_ROWS = 11  # ACT runs c1 of rows 0..this
ACT_ROWS = 40    # ACT multiply share upper bound (rows 12..38, i%4==2)
SWAP_RINGS = True  # wvb on scalar ring, hwg/whT/v3 on sync


def _build_nc() -> bacc.Bacc:
    nc = bacc.Bacc()

    wvb_d = nc.declare_dram_parameter("wvb", [P, KV * WVB], BF16, isOutput=False)
    hwg_d = nc.declare_dram_parameter("hwg", [P, KE + KE * E], BF16, isOutput=False)
    whT3_d = nc.declare_dram_parameter("whT3", [P, KE * IPC], BF16, isOutput=False)
    v3_d = nc.declare_dram_parameter("v3", [P, CJ * L], BF16, isOutput=False)
    out_d = nc.declare_dram_parameter("out", [IPC, L, L], BF16, isOutput=True)

    ring_wvb = nc.scalar if SWAP_RINGS else nc.sync
    ring_rest = nc.sync if SWAP_RINGS else nc.scalar

    with TileContext(nc) as tc:
        with (
            tc.tile_pool(name="const", bufs=1) as cpool,
            tc.tile_pool(name="work", bufs=2) as wpool,
            tc.tile_pool(name="psum", bufs=2, space="PSUM") as ppool,
            tc.tile_pool(name="outp", bufs=OUT_BUFS) as opool,
        ):
            # ---- input loads. wvb (s critical path) on its own ring,
            # chunked so s k0 can start while k1/k2 stream in.
            wvb_sb = cpool.tile([P, KV * WVB], BF16)
            for k in range(KV):
                ring_wvb.dma_start(
                    out=wvb_sb[:, k * WVB : (k + 1) * WVB],
                    in_=wvb_d[:, k * WVB : (k + 1) * WVB],
                )
            hwg_sb = cpool.tile([P, KE + KE * E], BF16)
            ring_rest.dma_start(out=hwg_sb[:], in_=hwg_d[:])
            whT_sb = cpool.tile([P, KE * IPC], BF16)
            ring_rest.dma_start(out=whT_sb[:], in_=whT3_d[:])
            v_sb = cpool.tile([P, CJ * L], BF16)
            ring_rest.dma_start(out=v_sb[:], in_=v3_d[:])

            h_sb = hwg_sb[:, 0:KE]
            wg_sb = hwg_sb[:, KE:]

            # Keep the PE busy from kernel start until the first input
            # chunk lands (HAM clock warmup; see module docstring).
            # One memset, 128-wide matmuls on the zeroed tile.
            warm_w = cpool.tile([P, P], BF16)
            nc.gpsimd.memset(warm_w[:], 0.0)
            warm_ps = ppool.tile([P, L], F32, tag="s_ps", bufs=KE)
            for w in range(NWARM):
                nc.tensor.matmul(
                    warm_ps[:, 0:P],
                    lhsT=warm_w[:],
                    rhs=warm_w[:],
                    start=(w == 0),
                    stop=(w == NWARM - 1),
                )

            ident = cpool.tile([IPC, IPC], F32)
            masks.make_identity(nc, ident[:])
            ones_row = cpool.tile([1, L], BF16)
            nc.gpsimd.memset(ones_row[:], 1.0)

            # ---- t = tanh(wv @ v + gh . 1^T), gh = wg @ h
            # t3[p, mc*384+j] = t[mc*128+p, j]
            t3 = cpool.tile([P, KE * L], BF16)
            s_ps = [
                ppool.tile([P, L], F32, tag="s_ps", bufs=KE, name=f"s_ps{mc}")
                for mc in range(KE)
            ]
            ghT_ps = ppool.tile([1, E], F32, tag="zg", bufs=2)

            def ghT_chunk(k):
                nc.tensor.matmul(
                    ghT_ps[:],
                    lhsT=h_sb[:, k : k + 1],
                    rhs=wg_sb[:, k * E : (k + 1) * E],
                    start=(k == 0),
                    stop=(k == KE - 1),
                )

            def s_pass(k, start):
                for mc in range(KE):
                    nc.tensor.matmul(
                        s_ps[mc][:],
                        lhsT=wvb_sb[:, k * WVB + mc * P : k * WVB + (mc + 1) * P],
                        rhs=wvb_sb[:, k * WVB + E : (k + 1) * WVB],
                        start=start,
                        stop=False,
                    )

            s_pass(0, start=True)
            for k in range(KE):
                ghT_chunk(k)
            s_pass(1, start=False)
            ghT_sb = wpool.tile([1, E], BF16)
            nc.vector.tensor_copy(ghT_sb[:], ghT_ps[:])

            def gh_close(mc):
                nc.tensor.matmul(
                    s_ps[mc][:],
                    lhsT=ghT_sb[:, mc * P : (mc + 1) * P],
                    rhs=ones_row[:],
                    start=False,
                    stop=True,
                )
                nc.scalar.activation(
                    t3[:, mc * L : (mc + 1) * L], s_ps[mc][:], AF.Tanh
                )

            # per-mc [k2, gh, tanh] so tanh0 fires right after its own
            # chunk closes instead of after the whole k2 pass
            k = KV - 1
            for mc in range(KE):
                nc.tensor.matmul(
                    s_ps[mc][:],
                    lhsT=wvb_sb[:, k * WVB + mc * P : k * WVB + (mc + 1) * P],
                    rhs=wvb_sb[:, k * WVB + E : (k + 1) * WVB],
                    start=False,
                    stop=False,
                )
                gh_close(mc)

            # ---- z rows, softmax (no max shift; fused row sums), and
            # alpha transpose.
            alphaT = wpool.tile([P, CJ * IPC], F32)

            from concourse.tile_rust import add_dep_helper

            def z_rows(r0, nr, after=None):
                z_h = ppool.tile([nr, L], F32, tag="zg", bufs=2)
                for kk in range(KE):
                    mm = nc.tensor.matmul(
                        z_h[:],
                        lhsT=whT_sb[:, kk * IPC + r0 : kk * IPC + r0 + nr],
                        rhs=t3[:, kk * L : (kk + 1) * L],
                        start=(kk == 0),
                        stop=(kk == KE - 1),
                    )
                    if kk == 0 and after is not None:
                        add_dep_helper(
                            mm.ins, after.ins, reason="z halves in order"
                        )
                e_h = wpool.tile([nr, L], F32, tag="e_h")
                rsum_h = wpool.tile([nr, 1], F32, tag="rsum_h")
                nc.scalar.activation(
                    e_h[:], z_h[:], AF.Exp, accum_out=rsum_h[:]
                )
                rinv_h = wpool.tile([nr, 1], F32, tag="rinv_h")
                nc.vector.reciprocal(rinv_h[:], rsum_h[:])
                # alphaT[p, c*48+i] = alpha[i, 3p+c]; the DVE normalize
                # also performs the stride-3 column gather (j = 3p+c) so
                # the PE transpose reads a contiguous slice.
                alpha_h = wpool.tile([nr, L], F32, tag="alpha_h")
                last_t = None
                for c in range(CJ):
                    nc.vector.tensor_scalar_mul(
                        alpha_h[:, c * P : (c + 1) * P],
                        e_h.rearrange("i (p c) -> c i p", c=CJ)[c],
                        rinv_h[:],
                    )
                    at_ps = ppool.tile([P, IPC if ZSPLIT == 1 else HZ],
                                       F32, tag="at_ps")
                    last_t = nc.tensor.transpose(
                        at_ps[:, 0:nr],
                        alpha_h[:, c * P : (c + 1) * P],
                        ident[0:nr, 0:nr],
                    )
                    nc.vector.tensor_copy(
                        alphaT[:, c * IPC + r0 : c * IPC + r0 + nr],
                        at_ps[:, 0:nr],
                    )
                return last_t

            def emit_block(ib, nb):
                ot = opool.tile([P, IPB * CJ * L], BF16, tag="ot")
                for t in range(nb):
                    i = ib + t
                    for c in range(CJ):
                        dst = ot[:, (t * CJ + c) * L : (t * CJ + c + 1) * L]
                        src = v_sb[:, c * L : (c + 1) * L]
                        sc = alphaT[:, c * IPC + i : c * IPC + i + 1]
                        if (i <= ACT_C1_ROWS and c == 1) or (
                            12 <= i < ACT_ROWS and i % 4 == 2
                        ):
                            # ACT runs c1 of the early rows and every
                            # 4th middle row so supply outpaces the
                            # stream
                            nc.scalar.mul(dst, src, sc)
                        else:
                            nc.vector.tensor_scalar_mul(dst, src, sc)
                # out row j = 3p+c -> 2304 B contiguous runs per (p, t)
                dram_ap = out_d[ib : ib + nb].rearrange(
                    "t (p c) l -> p t c l", p=P, c=CJ
                )
                sb_ap = ot[:, 0 : nb * CJ * L].rearrange(
                    "p (t c l) -> p t c l", t=nb, c=CJ
                )
                nc.sync.dma_start(out=dram_ap, in_=sb_ap)

            blocks = [(0, 1)]
            ib = 1
            while ib < IPC:
                nb = min(IPB, IPC - ib)
                blocks.append((ib, nb))
                ib += nb

            if ZSPLIT == 1:
                z_rows(0, IPC)
                for ib, nb in blocks:
                    emit_block(ib, nb)
            else:
                tr0 = z_rows(0, HZ)
                tr1 = None
                for ib, nb in blocks:
                    if ib >= ZH1_AT and tr1 is None:
                        tr1 = z_rows(HZ, HZ, after=tr0)
                    emit_block(ib, nb)

    nc.compile()
    return nc


def _prep_inputs(h, v, wh, wv, wg):
    """Host-side relayout into the per-core SBUF-friendly layouts."""
    import ml_dtypes

    h = np.ascontiguousarray(h, dtype=np.float32)
    v = np.ascontiguousarray(v, dtype=np.float32)
    wh = np.ascontiguousarray(wh, dtype=np.float32)
    wv = np.ascontiguousarray(wv, dtype=np.float32)
    wg = np.ascontiguousarray(wg, dtype=np.float32)

    def bf16(x):
        return np.ascontiguousarray(x.astype(ml_dtypes.bfloat16))

    # v3 (broadcast source): layout B, v3[p, c*384+l] = v[3p+c, l]
    v3 = bf16(v.reshape(P, CJ * L))
    # fused [wvT_k | vb_k] chunks: wvT_k[p, e] = wv[e, k*128+p],
    # vb_k[p, l] = v[k*128+p, l]
    wvT3 = wv.T.reshape(KV, P, E)
    vA = v.reshape(KV, P, L)
    wvb = bf16(
        np.concatenate(
            [np.concatenate([wvT3[k], vA[k]], axis=1) for k in range(KV)],
            axis=1,
        )
    )
    wgT3 = wg.T.reshape(KE, P, E).transpose(1, 0, 2).reshape(P, KE * E)
    hwg = bf16(np.concatenate([h.reshape(KE, P).T, wgT3], axis=1))

    in_maps = []
    for m in range(NCORES):
        whm = wh[m * IPC : (m + 1) * IPC]  # (48, 512)
        whT3 = bf16(
            whm.T.reshape(KE, P, IPC).transpose(1, 0, 2).reshape(P, KE * IPC)
        )
        in_maps.append(
            {
                "wvb": wvb,
                "hwg": hwg,
                "whT3": whT3,
                "v3": v3,
            }
        )
    return in_maps


_NC_CACHE = []


def _run(inputs: dict, trace: bool = False, **kw):
    if not _NC_CACHE:
        _NC_CACHE.append(_build_nc())
    nc = _NC_CACHE[0]
    in_maps = _prep_inputs(**inputs)
    res = run_bass_kernel_spmd(
        nc, in_maps, core_ids=list(range(NCORES)), trace=trace, **kw
    )
    out = np.concatenate(
        [r["out"].astype(np.float32) for r in res.results], axis=0
    )
    return out, res


def kernel(h, v, wh, wv, wg):
    out, _ = _run({"h": h, "v": v, "wh": wh, "wv": wv, "wg": wg})
    return out


# revision 10
# speedup vs baseline: 1.0322x; 1.0322x over previous
"""AttentionWeightedAverage distributed Trainium2 kernel.

Reference computation (all f32):
    s     = wv @ v + wg @ h          # (512, 384) + (512, 1) broadcast
    t     = tanh(s)                  # (512, 384)
    z     = wh @ t                   # (384, 384)
    alpha = softmax(z, axis=-1)      # (384, 384)
    out[i, j, l] = v[j, l] * alpha[i, j]   # (384, 384, 384)

The output (226 MB f32) dwarfs the inputs (~2.5 MB), so the kernel is
bound by per-core HBM write bandwidth. Sharding: every core gets the
full (small) weights and computes s/t redundantly; core m owns rows
i in [m*48, (m+1)*48) of z/alpha and writes that contiguous slice of
the output. No collectives.

Measured structure of a run (NTFF profile, exec_time = end of NEFF
postamble minus first useful instruction):
- ~1 us framework preamble inside the window (const memsets + engine
  barrier) - fixed.
- input loads at ~318 GB/s combined across the two HWDGE queues.
- compute chain wv@v -> tanh -> z -> softmax -> alphaT.
- ~38.6 us bf16 store stream at ~385 GB/s (the roofline).
- ~7.7 us fixed postamble: the NEFF epilogue zeroes all 253
  semaphores individually across the 5 engines - compiler-generated,
  not controllable from kernel code.

Design (v2; v1 was ~63.5 us, see git history of this docstring):
- Output stored as bf16, upcast to f32 on host (gate is 2e-2
  scale-relative; measured total err 5.7e-3). 14.2 MB/core.
- v3 broadcast source uses layout B (v3[p, c*384+l] = v[3p+c, l]) so
  each partition's 3 output rows are consecutive -> 2304 B contiguous
  HBM runs per store descriptor row.
- All weights bf16 (fp8 on anything feeding z fails the gate: sharp
  softmax amplifies logit noise - measured 6e-2).
- Load order: wvb chunks (the s critical path) go on the SCALAR ring,
  which the SDMA arbitration served first in every baseline trace;
  hwg/whT/v3 go on sync. In the baseline (wvb on sync) wvb's first
  byte trailed the scalar queue by ~2 us, putting wvb k2 at 13.6 us;
  swapped, k2 lands ~11 us.
- PE warmups: one 128x128 memset then NWARM 128-wide throwaway
  matmuls issued immediately at body start. The HAM clock gate needs
  ~3.4 us of sustained PE activity to double the PE clock and
  re-throttles after a ~2.3 us gap; the baseline's 7 warmups ended
  1.2 us before wvb k0 landed and the whole s chain ran at 1.2 GHz
  (throttle_active 46 us). Warmups sized to bridge until k0 lands.
- z/softmax in ONE 48-row pass (ZSPLIT=1): closes only ~0.2 us later
  than a 24-row half (matmul cost is rhs-width-bound) and removes the
  mid-stream half-1 softmax interruption entirely (the baseline spent
  real tuning effort placing it; not needed at all).
- softmax skips the max-subtraction: |z| stays far from f32 exp
  overflow and softmax is shift-invariant. The exp's accum_out gives
  the row sums for free.
- ALL store dma_starts on the sync ring as uniform 2-row blocks
  (block 0 is 1 row so its descgen is ~0.7 us). ACT runs c1 of rows
  0..ACT_C1_ROWS plus every 4th middle row; everything else on DVE
  (~229 ns per 128x384 bf16 tensor_scalar at 2x mode - the [P,1] f32
  scalar blocks 4x).
- Run-to-run NEFF time varies with HBM contention from the neighbor
  core (per-engine SDMA queue backlog can extend the tail; not
  fixable from the kernel).

Per-core SBUF layouts (P = 128 partitions):
    wvb  (128, 3*896) bf16: per k: [wvT_k | vb_k];
         wvT_k[p, e] = wv[e, k*128+p], vb_k[p, l] = v[k*128+p, l]
    hwg  (128, 4+2048) bf16: [h3 | wgT3]; h3[p,k] = h[k*128+p],
         wgT3[p, k*512+e] = wg[e, k*128+p]
    whT3 (128, 192) bf16: whT3[p, k*48+i] = wh[m*48+i, k*128+p]
    v3   (128, 1152) bf16: v3[p, c*384+l] = v[3p+c, l]
"""

import numpy as np

import concourse.bacc as bacc
import concourse.mybir as mybir
from concourse import masks
from concourse.bass_utils import run_bass_kernel_spmd
from concourse.tile import TileContext

F32 = mybir.dt.float32
BF16 = mybir.dt.bfloat16
AF = mybir.ActivationFunctionType

NCORES = 8
L = 384          # vfeat_len == vfeat_dim
E = 512          # embed dim
IPC = L // NCORES  # 48 output rows per core
P = 128
CJ = L // P      # 3 chunks over the j axis
KV = L // P      # 3 contraction chunks for wv@v
KE = E // P      # 4 contraction chunks over embed dim
WVB = E + L      # fused [wvT_k | vb_k] chunk width
IPB = 2          # output rows batched per store DMA
OUT_BUFS = 8     # in-flight output tiles
HZ = IPC // 2    # z/softmax half size (ZSPLIT=2 path)
NWARM = 9        # 128-wide PE warmups (bridge body start -> wvb k0 lands)
ZSPLIT = 2       # 1: one 48-row z/softmax pass; 2: two halves (h1 mid-stream)
ZH1_AT = 13      # (ZSPLIT=2 only) emit z half 1 before this block row
ACT_C1_ROWS = 11  # ACT runs c1 of rows 0..this
ACT_ROWS = 40    # ACT multiply share upper bound (rows 12..38, i%4==2)


def _build_nc() -> bacc.Bacc:
    nc = bacc.Bacc()

    wvb_d = nc.declare_dram_parameter("wvb", [P, KV * WVB], BF16, isOutput=False)
    hwg_d = nc.declare_dram_parameter("hwg", [P, KE + KE * E], BF16, isOutput=False)
    whT3_d = nc.declare_dram_parameter("whT3", [P, KE * IPC], BF16, isOutput=False)
    v3_d = nc.declare_dram_parameter("v3", [P, CJ * L], BF16, isOutput=False)
    out_d = nc.declare_dram_parameter("out", [IPC, L, L], BF16, isOutput=True)

    with TileContext(nc) as tc:
        with (
            tc.tile_pool(name="const", bufs=1) as cpool,
            tc.tile_pool(name="work", bufs=2) as wpool,
            tc.tile_pool(name="psum", bufs=2, space="PSUM") as ppool,
            tc.tile_pool(name="outp", bufs=OUT_BUFS) as opool,
        ):
            # ---- input loads, dual queue. The two HWDGE queues split
            # HBM read bandwidth roughly byte-proportionally and finish
            # together, so the split puts the s-critical wvb chunks
            # alone on sync and orders the scalar ring so only v3
            # (needed last, for the store multiplies) trails.
            wvb_sb = cpool.tile([P, KV * WVB], BF16)
            hwg_sb = cpool.tile([P, KE + KE * E], BF16)
            whT_sb = cpool.tile([P, KE * IPC], BF16)
            v_sb = cpool.tile([P, CJ * L], BF16)
            for k in range(KV):
                nc.sync.dma_start(
                    out=wvb_sb[:, k * WVB : (k + 1) * WVB],
                    in_=wvb_d[:, k * WVB : (k + 1) * WVB],
                )
            nc.scalar.dma_start(out=hwg_sb[:], in_=hwg_d[:])
            nc.scalar.dma_start(out=whT_sb[:], in_=whT3_d[:])
            nc.scalar.dma_start(out=v_sb[:], in_=v3_d[:])

            h_sb = hwg_sb[:, 0:KE]
            wg_sb = hwg_sb[:, KE:]

            # Keep the PE busy from kernel start until the first input
            # chunk lands (HAM clock warmup; see module docstring).
            # One memset, 128-wide matmuls on the zeroed tile.
            warm_w = cpool.tile([P, P], BF16)
            nc.gpsimd.memset(warm_w[:], 0.0)
            warm_ps = ppool.tile([P, L], F32, tag="s_ps", bufs=KE)
            for w in range(NWARM):
                nc.tensor.matmul(
                    warm_ps[:, 0:P],
                    lhsT=warm_w[:],
                    rhs=warm_w[:],
                    start=(w == 0),
                    stop=(w == NWARM - 1),
                )

            # The framework inserts the ACT table load right before the
            # first ACTIVATE in the ACT stream; a dummy 1-element
            # activation here pulls that 1.28 us load into the input
            # phase instead of the critical path before tanh0.
            dummy = cpool.tile([1, 1], F32)
            nc.gpsimd.memset(dummy[:], 0.0)
            nc.scalar.activation(dummy[:], dummy[:], AF.Tanh)

            ident = cpool.tile([IPC, IPC], F32)
            masks.make_identity(nc, ident[:])

            # ---- t = tanh(wv @ v + gh . 1^T), gh = wg @ h
            # t3[p, mc*384+j] = t[mc*128+p, j]
            # gh enters as the ACT bias of the tanh (func(in + bias)),
            # so the s PSUM closes directly off the k2 matmul and no
            # rank-1 gh update sits between k2 and tanh.
            t3 = cpool.tile([P, KE * L], BF16)
            s_ps = [
                ppool.tile([P, L], F32, tag="s_ps", bufs=KE, name=f"s_ps{mc}")
                for mc in range(KE)
            ]

            def s_pass(k, start, stop=False, tanh=False):
                for mc in range(KE):
                    nc.tensor.matmul(
                        s_ps[mc][:],
                        lhsT=wvb_sb[:, k * WVB + mc * P : k * WVB + (mc + 1) * P],
                        rhs=wvb_sb[:, k * WVB + E : (k + 1) * WVB],
                        start=start,
                        stop=stop,
                    )
                    if tanh:
                        nc.scalar.activation(
                            t3[:, mc * L : (mc + 1) * L],
                            s_ps[mc][:],
                            AF.Tanh,
                            bias=gh_sb[:, mc : mc + 1],
                        )

            s_pass(0, start=True)
            s_pass(1, start=False)

            # gh[mc*128+p] = sum_k wg[mc*128+p, k] h[k], computed
            # directly in partition-major layout so it can feed the
            # tanh bias without a transpose. 16 tiny matmuls; sits
            # between the s k1 and k2 passes, off the critical path.
            gh_ps = ppool.tile([P, KE], F32, tag="zg", bufs=2)
            for mc in range(KE):
                for k in range(KE):
                    nc.tensor.matmul(
                        gh_ps[:, mc : mc + 1],
                        lhsT=wg_sb[:, k * E + mc * P : k * E + (mc + 1) * P],
                        rhs=h_sb[:, k : k + 1],
                        start=(k == 0),
                        stop=(k == KE - 1),
                    )
            gh_sb = wpool.tile([P, KE], F32, tag="gh_sb")
            nc.vector.tensor_copy(gh_sb[:], gh_ps[:])

            s_pass(KV - 1, start=False, stop=True, tanh=True)

            # ---- z rows, softmax (no max shift; fused row sums), and
            # alpha transpose.
            alphaT = wpool.tile([P, CJ * IPC], F32)

            from concourse.tile_rust import add_dep_helper

            def z_rows(r0, nr, after=None):
                z_h = ppool.tile([nr, L], F32, tag="zg", bufs=2)
                for kk in range(KE):
                    mm = nc.tensor.matmul(
                        z_h[:],
                        lhsT=whT_sb[:, kk * IPC + r0 : kk * IPC + r0 + nr],
                        rhs=t3[:, kk * L : (kk + 1) * L],
                        start=(kk == 0),
                        stop=(kk == KE - 1),
                    )
                    if kk == 0 and after is not None:
                        add_dep_helper(
                            mm.ins, after.ins, reason="z halves in order"
                        )
                e_h = wpool.tile([nr, L], F32, tag="e_h")
                rsum_h = wpool.tile([nr, 1], F32, tag="rsum_h")
                nc.scalar.activation(
                    e_h[:], z_h[:], AF.Exp, accum_out=rsum_h[:]
                )
                rinv_h = wpool.tile([nr, 1], F32, tag="rinv_h")
                nc.vector.reciprocal(rinv_h[:], rsum_h[:])
                # alphaT[p, c*48+i] = alpha[i, 3p+c]; the DVE normalize
                # also performs the stride-3 column gather (j = 3p+c) so
                # the PE transpose reads a contiguous slice.  (The PE
                # transpose ignores its rhs operand's VALUES - it is a
                # pass-through mode - so the normalization cannot fold
                # into it via a diag(rinv) rhs; measured rel=1.9e3.)
                alpha_h = wpool.tile([nr, L], F32, tag="alpha_h")
                last_t = None
                for c in range(CJ):
                    nc.vector.tensor_scalar_mul(
                        alpha_h[:, c * P : (c + 1) * P],
                        e_h.rearrange("i (p c) -> c i p", c=CJ)[c],
                        rinv_h[:],
                    )
                    at_ps = ppool.tile([P, IPC if ZSPLIT == 1 else HZ],
                                       F32, tag="at_ps")
                    last_t = nc.tensor.transpose(
                        at_ps[:, 0:nr],
                        alpha_h[:, c * P : (c + 1) * P],
                        ident[0:nr, 0:nr],
                    )
                    # PSUM->SBUF copies on ACT (idle here) so the DVE
                    # only runs the three gathers back to back
                    nc.scalar.activation(
                        alphaT[:, c * IPC + r0 : c * IPC + r0 + nr],
                        at_ps[:, 0:nr],
                        AF.Copy,
                    )
                return last_t

            def emit_block(ib, nb):
                ot = opool.tile([P, IPB * CJ * L], BF16, tag="ot")
                for t in range(nb):
                    i = ib + t
                    for c in range(CJ):
                        dst = ot[:, (t * CJ + c) * L : (t * CJ + c + 1) * L]
                        src = v_sb[:, c * L : (c + 1) * L]
                        sc = alphaT[:, c * IPC + i : c * IPC + i + 1]
                        if (3 <= i <= ACT_C1_ROWS and c == 1) or (
                            12 <= i < ACT_ROWS and i % 4 == 2
                        ):
                            # ACT runs c1 of the early rows and every
                            # 4th middle row so supply outpaces the
                            # stream
                            nc.scalar.mul(dst, src, sc)
                        else:
                            nc.vector.tensor_scalar_mul(dst, src, sc)
                # out row j = 3p+c -> 2304 B contiguous runs per (p, t)
                dram_ap = out_d[ib : ib + nb].rearrange(
                    "t (p c) l -> p t c l", p=P, c=CJ
                )
                sb_ap = ot[:, 0 : nb * CJ * L].rearrange(
                    "p (t c l) -> p t c l", t=nb, c=CJ
                )
                nc.sync.dma_start(out=dram_ap, in_=sb_ap)

            blocks = [(0, 1)]
            ib = 1
            while ib < IPC:
                nb = min(IPB, IPC - ib)
                blocks.append((ib, nb))
                ib += nb

            if ZSPLIT == 1:
                z_rows(0, IPC)
                for ib, nb in blocks:
                    emit_block(ib, nb)
            else:
                tr0 = z_rows(0, HZ)
                tr1 = None
                for ib, nb in blocks:
                    if ib >= ZH1_AT and tr1 is None:
                        tr1 = z_rows(HZ, HZ, after=tr0)
                    emit_block(ib, nb)

    nc.compile()
    return nc


def _prep_inputs(h, v, wh, wv, wg):
    """Host-side relayout into the per-core SBUF-friendly layouts."""
    import ml_dtypes

    h = np.ascontiguousarray(h, dtype=np.float32)
    v = np.ascontiguousarray(v, dtype=np.float32)
    wh = np.ascontiguousarray(wh, dtype=np.float32)
    wv = np.ascontiguousarray(wv, dtype=np.float32)
    wg = np.ascontiguousarray(wg, dtype=np.float32)

    def bf16(x):
        return np.ascontiguousarray(x.astype(ml_dtypes.bfloat16))

    # v3 (broadcast source): layout B, v3[p, c*384+l] = v[3p+c, l]
    v3 = bf16(v.reshape(P, CJ * L))
    # fused [wvT_k | vb_k] chunks: wvT_k[p, e] = wv[e, k*128+p],
    # vb_k[p, l] = v[k*128+p, l]
    wvT3 = wv.T.reshape(KV, P, E)
    vA = v.reshape(KV, P, L)
    wvb = bf16(
        np.concatenate(
            [np.concatenate([wvT3[k], vA[k]], axis=1) for k in range(KV)],
            axis=1,
        )
    )
    wgT3 = wg.T.reshape(KE, P, E).transpose(1, 0, 2).reshape(P, KE * E)
    hwg = bf16(np.concatenate([h.reshape(KE, P).T, wgT3], axis=1))

    in_maps = []
    for m in range(NCORES):
        whm = wh[m * IPC : (m + 1) * IPC]  # (48, 512)
        whT3 = bf16(
            whm.T.reshape(KE, P, IPC).transpose(1, 0, 2).reshape(P, KE * IPC)
        )
        in_maps.append(
            {
                "wvb": wvb,
                "hwg": hwg,
                "whT3": whT3,
                "v3": v3,
            }
        )
    return in_maps


_NC_CACHE = []


def _run(inputs: dict, trace: bool = False, **kw):
    if not _NC_CACHE:
        _NC_CACHE.append(_build_nc())
    nc = _NC_CACHE[0]
    in_maps = _prep_inputs(**inputs)
    res = run_bass_kernel_spmd(
        nc, in_maps, core_ids=list(range(NCORES)), trace=trace, **kw
    )
    out = np.concatenate(
        [r["out"].astype(np.float32) for r in res.results], axis=0
    )
    return out, res


def kernel(h, v, wh, wv, wg):
    out, _ = _run({"h": h, "v": v, "wh": wh, "wv": wv, "wg": wg})
    return out


# revision 12
# speedup vs baseline: 1.1128x; 1.0780x over previous
"""AttentionWeightedAverage distributed Trainium2 kernel.

Reference computation (all f32):
    s     = wv @ v + wg @ h          # (512, 384) + (512, 1) broadcast
    t     = tanh(s)                  # (512, 384)
    z     = wh @ t                   # (384, 384)
    alpha = softmax(z, axis=-1)      # (384, 384)
    out[i, j, l] = v[j, l] * alpha[i, j]   # (384, 384, 384)

The output (226 MB f32) dwarfs the inputs (~2.5 MB), so the kernel is
bound by per-core HBM write bandwidth. Sharding: every core gets the
full (small) weights and computes s/t redundantly; core m owns rows
i in [m*48, (m+1)*48) of z/alpha and writes that contiguous slice of
the output. No collectives.

Measured structure of a run (NTFF profile, exec_time = end of NEFF
postamble minus first useful instruction):
- ~1 us framework preamble inside the window (const memsets + engine
  barrier) - fixed.
- input loads at ~318 GB/s combined across the two HWDGE queues.
- compute chain wv@v -> tanh -> z -> softmax -> alphaT.
- ~38.6 us bf16 store stream at ~385 GB/s (the roofline).
- ~7.7 us fixed postamble: the NEFF epilogue zeroes all 253
  semaphores individually across the 5 engines - compiler-generated,
  not controllable from kernel code.

Design (v2; v1 was ~63.5 us, see git history of this docstring):
- Output stored as bf16, upcast to f32 on host (gate is 2e-2
  scale-relative; measured total err 5.7e-3). 14.2 MB/core.
- v3 broadcast source uses layout B (v3[p, c*384+l] = v[3p+c, l]) so
  each partition's 3 output rows are consecutive -> 2304 B contiguous
  HBM runs per store descriptor row.
- All weights bf16 (fp8 on anything feeding z fails the gate: sharp
  softmax amplifies logit noise - measured 6e-2).
- Load order: wvb chunks (the s critical path) go on the SCALAR ring,
  which the SDMA arbitration served first in every baseline trace;
  hwg/whT/v3 go on sync. In the baseline (wvb on sync) wvb's first
  byte trailed the scalar queue by ~2 us, putting wvb k2 at 13.6 us;
  swapped, k2 lands ~11 us.
- PE warmups: one 128x128 memset then NWARM 128-wide throwaway
  matmuls issued immediately at body start. The HAM clock gate needs
  ~3.4 us of sustained PE activity to double the PE clock and
  re-throttles after a ~2.3 us gap; the baseline's 7 warmups ended
  1.2 us before wvb k0 landed and the whole s chain ran at 1.2 GHz
  (throttle_active 46 us). Warmups sized to bridge until k0 lands.
- z/softmax in ONE 48-row pass (ZSPLIT=1): closes only ~0.2 us later
  than a 24-row half (matmul cost is rhs-width-bound) and removes the
  mid-stream half-1 softmax interruption entirely (the baseline spent
  real tuning effort placing it; not needed at all).
- softmax skips the max-subtraction: |z| stays far from f32 exp
  overflow and softmax is shift-invariant. The exp's accum_out gives
  the row sums for free.
- ALL store dma_starts on the sync ring as uniform 2-row blocks
  (block 0 is 1 row so its descgen is ~0.7 us). ACT runs c1 of rows
  0..ACT_C1_ROWS plus every 4th middle row; everything else on DVE
  (~229 ns per 128x384 bf16 tensor_scalar at 2x mode - the [P,1] f32
  scalar blocks 4x).
- Run-to-run NEFF time varies with HBM contention from the neighbor
  core (per-engine SDMA queue backlog can extend the tail; not
  fixable from the kernel).

Per-core SBUF layouts (P = 128 partitions):
    wvb  (128, 3*896) bf16: per k: [wvT_k | vb_k];
         wvT_k[p, e] = wv[e, k*128+p], vb_k[p, l] = v[k*128+p, l]
    hwg  (128, 4+2048) bf16: [h3 | wgT3]; h3[p,k] = h[k*128+p],
         wgT3[p, k*512+e] = wg[e, k*128+p]
    whT3 (128, 192) bf16: whT3[p, k*48+i] = wh[m*48+i, k*128+p]
    v3   (128, 1152) bf16: v3[p, c*384+l] = v[3p+c, l]
"""

import numpy as np

import concourse.bacc as bacc
import concourse.mybir as mybir
from concourse import masks
from concourse.bass_utils import run_bass_kernel_spmd
from concourse.tile import TileContext

F32 = mybir.dt.float32
BF16 = mybir.dt.bfloat16
AF = mybir.ActivationFunctionType

NCORES = 8
L = 384          # vfeat_len == vfeat_dim
E = 512          # embed dim
IPC = L // NCORES  # 48 output rows per core
P = 128
CJ = L // P      # 3 chunks over the j axis
KV = L // P      # 3 contraction chunks for wv@v
KE = E // P      # 4 contraction chunks over embed dim
WVB = E + L      # fused [wvT_k | vb_k] chunk width
IPB = 2          # output rows batched per store DMA
OUT_BUFS = 8     # in-flight output tiles
HZ = IPC // 2    # z/softmax half size (ZSPLIT=2 path)
NWARM = 9        # 128-wide PE warmups (bridge body start -> wvb k0 lands)
ZSPLIT = 2       # 1: one 48-row z/softmax pass; 2: two halves (h1 mid-stream)
ZH1_AT = 13      # (ZSPLIT=2 only) emit z half 1 before this block row
ACT_C1_ROWS = 11  # ACT runs c1 of rows 0..this
ACT_ROWS = 40    # ACT multiply share upper bound (rows 12..38, i%4==2)


def _build_nc() -> bacc.Bacc:
    nc = bacc.Bacc()

    wvb_d = nc.declare_dram_parameter("wvb", [P, KV * WVB], BF16, isOutput=False)
    hwg_d = nc.declare_dram_parameter("hwg", [P, KE + KE * E], BF16, isOutput=False)
    whT3_d = nc.declare_dram_parameter("whT3", [P, KE * IPC], BF16, isOutput=False)
    v3_d = nc.declare_dram_parameter("v3", [P, CJ * L], BF16, isOutput=False)
    out_d = nc.declare_dram_parameter("out", [IPC, L, L], BF16, isOutput=True)

    with TileContext(nc) as tc:
        with (
            tc.tile_pool(name="const", bufs=1) as cpool,
            tc.tile_pool(name="work", bufs=2) as wpool,
            tc.tile_pool(name="psum", bufs=2, space="PSUM") as ppool,
            tc.tile_pool(name="outp", bufs=OUT_BUFS) as opool,
        ):
            # ---- input loads: ALL on the scalar (ACT) HWDGE queue in
            # dependency order. Measured: when both HWDGE queues carry
            # input work the scalar queue transfers ~1.9 us before the
            # sync queue joins, and two live queues split HBM bandwidth
            # anyway, so one early queue in exact FIFO dependency order
            # beats any dual-queue split. The sync queue stays empty
            # for the store stream descgen.
            wvb_sb = cpool.tile([P, KV * WVB], BF16)
            hwg_sb = cpool.tile([P, KE + KE * E], BF16)
            whT_sb = cpool.tile([P, KE * IPC], BF16)
            v_sb = cpool.tile([P, CJ * L], BF16)
            for k in range(KV - 1):
                nc.scalar.dma_start(
                    out=wvb_sb[:, k * WVB : (k + 1) * WVB],
                    in_=wvb_d[:, k * WVB : (k + 1) * WVB],
                )
            nc.scalar.dma_start(out=hwg_sb[:], in_=hwg_d[:])
            k = KV - 1
            nc.scalar.dma_start(
                out=wvb_sb[:, k * WVB : (k + 1) * WVB],
                in_=wvb_d[:, k * WVB : (k + 1) * WVB],
            )
            nc.scalar.dma_start(out=whT_sb[:], in_=whT3_d[:])
            nc.scalar.dma_start(out=v_sb[:], in_=v3_d[:])

            h_sb = hwg_sb[:, 0:KE]
            wg_sb = hwg_sb[:, KE:]

            # Keep the PE busy from kernel start until the first input
            # chunk lands (HAM clock warmup; see module docstring).
            # One memset, 128-wide matmuls on the zeroed tile.
            warm_w = cpool.tile([P, P], BF16)
            nc.gpsimd.memset(warm_w[:], 0.0)
            warm_ps = ppool.tile([P, L], F32, tag="s_ps", bufs=KE)
            for w in range(NWARM):
                nc.tensor.matmul(
                    warm_ps[:, 0:P],
                    lhsT=warm_w[:],
                    rhs=warm_w[:],
                    start=(w == 0),
                    stop=(w == NWARM - 1),
                )

            # The framework inserts the ACT table load right before the
            # first ACTIVATE in the ACT stream; a dummy 1-element
            # activation here pulls that 1.28 us load into the input
            # phase instead of the critical path before tanh0.
            dummy = cpool.tile([1, 1], F32)
            nc.gpsimd.memset(dummy[:], 0.0)
            nc.scalar.activation(dummy[:], dummy[:], AF.Tanh)

            ident = cpool.tile([IPC, IPC], F32)
            masks.make_identity(nc, ident[:])

            # ---- t = tanh(wv @ v + gh . 1^T), gh = wg @ h
            # t3[p, mc*384+j] = t[mc*128+p, j]
            # gh enters as the ACT bias of the tanh (func(in + bias)),
            # so the s PSUM closes directly off the k2 matmul and no
            # rank-1 gh update sits between k2 and tanh.
            t3 = cpool.tile([P, KE * L], BF16)
            s_ps = [
                ppool.tile([P, L], F32, tag="s_ps", bufs=KE, name=f"s_ps{mc}")
                for mc in range(KE)
            ]

            # The tanh reads f32 PSUM at 1 elem/cycle (580 ns per mc);
            # staging s through a DVE bf16 SBUF copy (325 ns, DVE idle
            # here) lets the ACT run at 2x (347 ns) and the two
            # pipeline across engines. Costs ~0.4% extra noise on s,
            # inside the error budget (measured 5.7e-3 vs 2e-2 gate).
            s_bf = cpool.tile([P, KE * L], BF16, name="s_bf")

            def s_pass(k, start, stop=False, tanh=False):
                for mc in range(KE):
                    nc.tensor.matmul(
                        s_ps[mc][:],
                        lhsT=wvb_sb[:, k * WVB + mc * P : k * WVB + (mc + 1) * P],
                        rhs=wvb_sb[:, k * WVB + E : (k + 1) * WVB],
                        start=start,
                        stop=stop,
                    )
                    if tanh:
                        nc.vector.tensor_copy(
                            s_bf[:, mc * L : (mc + 1) * L], s_ps[mc][:]
                        )
                        nc.scalar.activation(
                            t3[:, mc * L : (mc + 1) * L],
                            s_bf[:, mc * L : (mc + 1) * L],
                            AF.Tanh,
                            bias=gh_sb[:, mc : mc + 1],
                        )

            s_pass(0, start=True)
            s_pass(1, start=False)

            # gh[mc*128+p] = sum_k wg[mc*128+p, k] h[k], computed
            # directly in partition-major layout so it can feed the
            # tanh bias without a transpose. 16 tiny matmuls; sits
            # between the s k1 and k2 passes, off the critical path.
            gh_ps = ppool.tile([P, KE], F32, tag="zg", bufs=2)
            for mc in range(KE):
                for k in range(KE):
                    nc.tensor.matmul(
                        gh_ps[:, mc : mc + 1],
                        lhsT=wg_sb[:, k * E + mc * P : k * E + (mc + 1) * P],
                        rhs=h_sb[:, k : k + 1],
                        start=(k == 0),
                        stop=(k == KE - 1),
                    )
            gh_sb = wpool.tile([P, KE], F32, tag="gh_sb")
            nc.vector.tensor_copy(gh_sb[:], gh_ps[:])

            s_pass(KV - 1, start=False, stop=True, tanh=True)

            # ---- z rows, softmax (no max shift; fused row sums), and
            # alpha transpose.
            alphaT = wpool.tile([P, CJ * IPC], F32)

            from concourse.tile_rust import add_dep_helper

            def z_rows(r0, nr, after=None):
                z_h = ppool.tile([nr, L], F32, tag="zg", bufs=2)
                for kk in range(KE):
                    mm = nc.tensor.matmul(
                        z_h[:],
                        lhsT=whT_sb[:, kk * IPC + r0 : kk * IPC + r0 + nr],
                        rhs=t3[:, kk * L : (kk + 1) * L],
                        start=(kk == 0),
                        stop=(kk == KE - 1),
                    )
                    if kk == 0 and after is not None:
                        add_dep_helper(
                            mm.ins, after.ins, reason="z halves in order"
                        )
                e_h = wpool.tile([nr, L], F32, tag="e_h")
                rsum_h = wpool.tile([nr, 1], F32, tag="rsum_h")
                nc.scalar.activation(
                    e_h[:], z_h[:], AF.Exp, accum_out=rsum_h[:]
                )
                rinv_h = wpool.tile([nr, 1], F32, tag="rinv_h")
                nc.vector.reciprocal(rinv_h[:], rsum_h[:])
                # alphaT[p, c*48+i] = alpha[i, 3p+c]; the DVE normalize
                # also performs the stride-3 column gather (j = 3p+c) so
                # the PE transpose reads a contiguous slice.  (The PE
                # transpose ignores its rhs operand's VALUES - it is a
                # pass-through mode - so the normalization cannot fold
                # into it via a diag(rinv) rhs; measured rel=1.9e3.)
                alpha_h = wpool.tile([nr, L], F32, tag="alpha_h")
                last_t = None
                for c in range(CJ):
                    nc.vector.tensor_scalar_mul(
                        alpha_h[:, c * P : (c + 1) * P],
                        e_h.rearrange("i (p c) -> c i p", c=CJ)[c],
                        rinv_h[:],
                    )
                    at_ps = ppool.tile([P, IPC if ZSPLIT == 1 else HZ],
                                       F32, tag="at_ps")
                    last_t = nc.tensor.transpose(
                        at_ps[:, 0:nr],
                        alpha_h[:, c * P : (c + 1) * P],
                        ident[0:nr, 0:nr],
                    )
                    # PSUM->SBUF copies on ACT (idle here) so the DVE
                    # only runs the three gathers back to back
                    nc.scalar.activation(
                        alphaT[:, c * IPC + r0 : c * IPC + r0 + nr],
                        at_ps[:, 0:nr],
                        AF.Copy,
                    )
                return last_t

            def emit_block(ib, nb):
                ot = opool.tile([P, IPB * CJ * L], BF16, tag="ot")
                for t in range(nb):
                    i = ib + t
                    for c in range(CJ):
                        dst = ot[:, (t * CJ + c) * L : (t * CJ + c + 1) * L]
                        src = v_sb[:, c * L : (c + 1) * L]
                        sc = alphaT[:, c * IPC + i : c * IPC + i + 1]
                        if (3 <= i <= ACT_C1_ROWS and c == 1) or (
                            12 <= i < ACT_ROWS and i % 4 == 2
                        ):
                            # ACT runs c1 of the early rows and every
                            # 4th middle row so supply outpaces the
                            # stream
                            nc.scalar.mul(dst, src, sc)
                        else:
                            nc.vector.tensor_scalar_mul(dst, src, sc)
                # out row j = 3p+c -> 2304 B contiguous runs per (p, t)
                dram_ap = out_d[ib : ib + nb].rearrange(
                    "t (p c) l -> p t c l", p=P, c=CJ
                )
                sb_ap = ot[:, 0 : nb * CJ * L].rearrange(
                    "p (t c l) -> p t c l", t=nb, c=CJ
                )
                nc.sync.dma_start(out=dram_ap, in_=sb_ap)

            blocks = [(0, 1)]
            ib = 1
            while ib < IPC:
                nb = min(IPB, IPC - ib)
                blocks.append((ib, nb))
                ib += nb

            if ZSPLIT == 1:
                z_rows(0, IPC)
                for ib, nb in blocks:
                    emit_block(ib, nb)
            else:
                tr0 = z_rows(0, HZ)
                tr1 = None
                for ib, nb in blocks:
                    if ib >= ZH1_AT and tr1 is None:
                        tr1 = z_rows(HZ, HZ, after=tr0)
                    emit_block(ib, nb)

    nc.compile()
    return nc


def _prep_inputs(h, v, wh, wv, wg):
    """Host-side relayout into the per-core SBUF-friendly layouts."""
    import ml_dtypes

    h = np.ascontiguousarray(h, dtype=np.float32)
    v = np.ascontiguousarray(v, dtype=np.float32)
    wh = np.ascontiguousarray(wh, dtype=np.float32)
    wv = np.ascontiguousarray(wv, dtype=np.float32)
    wg = np.ascontiguousarray(wg, dtype=np.float32)

    def bf16(x):
        return np.ascontiguousarray(x.astype(ml_dtypes.bfloat16))

    # v3 (broadcast source): layout B, v3[p, c*384+l] = v[3p+c, l]
    v3 = bf16(v.reshape(P, CJ * L))
    # fused [wvT_k | vb_k] chunks: wvT_k[p, e] = wv[e, k*128+p],
    # vb_k[p, l] = v[k*128+p, l]
    wvT3 = wv.T.reshape(KV, P, E)
    vA = v.reshape(KV, P, L)
    wvb = bf16(
        np.concatenate(
            [np.concatenate([wvT3[k], vA[k]], axis=1) for k in range(KV)],
            axis=1,
        )
    )
    wgT3 = wg.T.reshape(KE, P, E).transpose(1, 0, 2).reshape(P, KE * E)
    hwg = bf16(np.concatenate([h.reshape(KE, P).T, wgT3], axis=1))

    in_maps = []
    for m in range(NCORES):
        whm = wh[m * IPC : (m + 1) * IPC]  # (48, 512)
        whT3 = bf16(
            whm.T.reshape(KE, P, IPC).transpose(1, 0, 2).reshape(P, KE * IPC)
        )
        in_maps.append(
            {
                "wvb": wvb,
                "hwg": hwg,
                "whT3": whT3,
                "v3": v3,
            }
        )
    return in_maps


_NC_CACHE = []


def _run(inputs: dict, trace: bool = False, **kw):
    if not _NC_CACHE:
        _NC_CACHE.append(_build_nc())
    nc = _NC_CACHE[0]
    in_maps = _prep_inputs(**inputs)
    res = run_bass_kernel_spmd(
        nc, in_maps, core_ids=list(range(NCORES)), trace=trace, **kw
    )
    out = np.concatenate(
        [r["out"].astype(np.float32) for r in res.results], axis=0
    )
    return out, res


def kernel(h, v, wh, wv, wg):
    out, _ = _run({"h": h, "v": v, "wh": wh, "wv": wv, "wg": wg})
    return out
